# revision 3
# baseline (speedup 1.0000x reference)
"""BatchChildSumTreeLSTM Trainium2 kernel, v2.

Forest of T complete B-ary trees, level-major layout, leaves-first.
Sharded across 8 NeuronCores by tree (2 trees/core); recursion is
device-local.

v2 changes vs baseline:
  * `xb` (parent-embed-per-child) input eliminated: the f-gate's fx
    matmul streams the PARENT xT slice through the PE with a
    column-repeat (stride-0) access pattern, so no extra DRAM traffic
    or host-side np.repeat.
  * Parent-level xT (levels 0..depth-2, 10922 cols/core) resident in
    SBUF via one up-front DMA; only the leaf level streams in chunks.
  * All elementwise tensors bf16 (incl. cell state c): every VectorE
    tensor_tensor runs in 2x_1P packed mode, roughly halving DVE busy
    time vs the f32 baseline.  PSUM stays f32; per-gate biases applied
    in fp32 by ScalarE's free affine.
"""

import sys

if "/opt/trn_rl_repo" not in sys.path:
    sys.path.insert(0, "/opt/trn_rl_repo")

import numpy as np

P = 128          # feature dim == partitions
BR = 4           # branching factor
NLBL = 5

_NC_CACHE = {}


def _levels(tpc, depth):
    n = [tpc * BR**l for l in range(depth)]
    off = [0]
    for c in n:
        off.append(off[-1] + c)
    return n, off, off[-1]


def _legalize_waits(nc, max_waits=1):
    """This walrus build accepts at most one sync-wait command per
    instruction (any type).  Hoist excess waits onto same-engine NoOps
    inserted right before the instruction; engine program order makes
    this exactly equivalent."""
    import concourse.mybir as mybir

    n_nops = 0
    for fn in nc.m.functions:
        for blk in fn.blocks:
            new_insts = []
            for inst in blk.instructions:
                si = getattr(inst, "sync_info", None)
                if si is not None and si.on_wait and len(si.on_wait) > max_waits:
                    waits = list(si.on_wait)
                    hoist, keep = waits[:-max_waits], waits[-max_waits:]
                    eng = getattr(inst, "engine", None)
                    for j, w in enumerate(hoist):
                        nop = mybir.InstNoOp(
                            name=f"{inst.name}-wn{j}",
                            engine=eng,
                            bass_nofuse=True,
                        )
                        nop.sync_info = mybir.SyncInfo(on_wait=[w],
                                                       on_update=[])
                        new_insts.append(nop)
                        n_nops += 1
                    inst.sync_info = mybir.SyncInfo(
                        on_wait=keep, on_update=list(si.on_update))
                new_insts.append(inst)
            blk.instructions = new_insts
    return n_nops


def build_nc(tpc=2, depth=8, ch_leaf=2048, nlbl=NLBL, legalize=True):
    """Build the per-core Bass/Tile program (identical on all cores)."""
    import concourse.bass as bass
    import concourse.mybir as mybir
    import concourse.tile as tile

    f32 = mybir.dt.float32
    bf16 = mybir.dt.bfloat16
    AF = mybir.ActivationFunctionType
    ADD = mybir.AluOpType.add

    n, off, ntot = _levels(tpc, depth)
    lleaf = depth - 1
    nleaf = n[lleaf]
    npar_tot = off[lleaf]          # nodes in levels 0..depth-2
    ch_leaf = min(ch_leaf, nleaf)
    assert nleaf % ch_leaf == 0

    nc = bass.Bass()

    xT = nc.dram_tensor("xT", [P, ntot], bf16, kind="ExternalInput")
    wnames = ["ix", "ih", "fx", "fh", "ox", "oh", "ux", "uh"]
    wall = nc.dram_tensor("Wall", [P, len(wnames), P], bf16,
                          kind="ExternalInput")
    bias4 = nc.dram_tensor("bias4", [P, 4], f32, kind="ExternalInput")
    woutT = nc.dram_tensor("WoutT", [P, nlbl], bf16, kind="ExternalInput")
    bout = nc.dram_tensor("bout2", [tpc, nlbl], f32, kind="ExternalInput")
    out = nc.dram_tensor("out", [tpc, nlbl], f32, kind="ExternalOutput")

    assert depth >= 4

    SIG, TANH = AF.Sigmoid, AF.Tanh
    BIDX = {"i": 0, "f": 1, "o": 2, "u": 3}

    lp = nc.allow_low_precision("bf16 LSTM cell state; tol 2e-2")
    lp.__enter__()
    with tile.TileContext(nc) as tc:
        import contextlib
        with contextlib.ExitStack() as ctx:
            wp = ctx.enter_context(tc.tile_pool(name="wp", bufs=1))
            stream = ctx.enter_context(tc.tile_pool(name="stream", bufs=2))
            tmp = ctx.enter_context(tc.tile_pool(name="tmp", bufs=2))
            acc = ctx.enter_context(tc.tile_pool(name="acc", bufs=2))
            chk = ctx.enter_context(tc.tile_pool(name="chk", bufs=2))
            hcp = ctx.enter_context(tc.tile_pool(name="hcp", bufs=1))
            psum = ctx.enter_context(
                tc.tile_pool(name="psum", bufs=2, space="PSUM"))

            GMAX = 2048  # max columns per gate/f group (4 PSUM banks)

            # ---- load constants; parent embeds are DMA'd after the first
            # leaf chunks so the leaf pipeline starts immediately ----
            wall_sb = wp.tile([P, len(wnames), P], bf16, name="wall_sb",
                              tag="wall_sb")
            nc.sync.dma_start(out=wall_sb, in_=wall[:])
            wsb = {nm: wall_sb[:, j, :] for j, nm in enumerate(wnames)}
            bias_sb = wp.tile([P, 4], f32, name="bias_sb", tag="bias_sb")
            nc.sync.dma_start(out=bias_sb, in_=bias4[:])
            woutT_sb = wp.tile([P, nlbl], bf16, name="woutT_sb", tag="woutT_sb")
            bout_sb = wp.tile([tpc, nlbl], f32, name="bout_sb", tag="bout_sb")
            xtp = wp.tile([P, npar_tot], bf16, name="xtp", tag="xtp")

            def load_parent_embeds(l6_off):
                # level depth-2 slice first (needed by the first leaf
                # f-pass), the small upper levels + output consts after
                nc.sync.dma_start(out=xtp[:, l6_off:npar_tot],
                                  in_=xT[:, l6_off:npar_tot])
                nc.sync.dma_start(out=xtp[:, 0:l6_off], in_=xT[:, 0:l6_off])
                nc.sync.dma_start(out=woutT_sb, in_=woutT[:])
                nc.sync.dma_start(out=bout_sb, in_=bout[:])

            def bias_ap(g):
                i = BIDX[g]
                return bias_sb[:, i:i + 1]

            def xpar_bcast(l, a, m):
                """Parent-embed columns [a, a+m) of level l, each column
                repeated BR times -> free size m*BR (the on-chip `xb`)."""
                base = xtp[:, off[l] + a:off[l] + a + m]
                return base.unsqueeze(2).broadcast_to((P, m, BR))

            # persistent h/c for resident levels (leaves and level
            # depth-2 are streamed/consumed in flight, never stored whole)
            hres, cres = {}, {}
            for l in range(depth - 2):
                hres[l] = hcp.tile([P, n[l]], bf16, name=f"h{l}_sb",
                                   tag=f"h{l}_sb")
                cres[l] = hcp.tile([P, n[l]], bf16, name=f"c{l}_sb",
                                   tag=f"c{l}_sb")

            def matmul_group(ps, w0, rhs0, w1=None, rhs1=None, G=GMAX):
                """ps[:, :G] = w0.T@rhs0 (+ w1.T@rhs1).  512-col banks."""
                nb = (G + 511) // 512
                for b in range(nb):
                    s = b * 512
                    e = min(s + 512, G)
                    nc.tensor.matmul(ps[:, s:e], wsb[w0],
                                     _slice_free(rhs0, s, e),
                                     start=True, stop=(w1 is None))
                if w1 is not None:
                    for b in range(nb):
                        s = b * 512
                        e = min(s + 512, G)
                        nc.tensor.matmul(ps[:, s:e], wsb[w1],
                                         _slice_free(rhs1, s, e),
                                         start=False, stop=True)

            def _slice_free(ap, s, e):
                """Slice [s, e) of the flattened free dim of a 2d or
                broadcast 3d AP (whose last dim is the BR repeat)."""
                if len(ap.shape) == 2:
                    return ap[:, s:e]
                assert s % BR == 0 and e % BR == 0
                return ap[:, s // BR:e // BR, :]

            # minimax cubic for tanh on [-1,1] (leaf c = i*u is always in
            # (-1,1)); end-to-end error is indistinguishable from exact
            # tanh at bf16 precision, and it moves leaf tanh off ScalarE
            TANH3_A = -0.2487
            MULT = mybir.AluOpType.mult
            ADDOP = mybir.AluOpType.add

            def gate_pass(xt_ap, hs_ap, fcs_ap, h_out, c_out, G,
                          leaf_dve_tanh=True):
                """Compute i,u,o gates + c,h for G parent columns."""
                leaf = hs_ap is None
                g_sb = {}
                for gname, wx, wh, func in (("i", "ix", "ih", SIG),
                                            ("u", "ux", "uh", TANH),
                                            ("o", "ox", "oh", SIG)):
                    ps = psum.tile([P, GMAX], f32, name=f"ps_{gname}",
                                   tag="ps")
                    if leaf:
                        matmul_group(ps, wx, xt_ap, G=G)
                    else:
                        matmul_group(ps, wx, xt_ap, wh, hs_ap, G=G)
                    g = tmp.tile([P, G], bf16, name=f"g_{gname}",
                                 tag=f"g_{gname}")
                    nc.scalar.activation(out=g, in_=ps[:, :G], func=func,
                                         bias=bias_ap(gname), scale=1.0)
                    g_sb[gname] = g
                nc.vector.tensor_mul(c_out, g_sb["i"], g_sb["u"])
                if fcs_ap is not None:
                    nc.vector.tensor_add(c_out, c_out, fcs_ap)
                if leaf and leaf_dve_tanh:
                    # h = (o*c)*(1 + A*c^2) on VectorE (ScalarE is the
                    # kernel-wide bottleneck; |c|<1 so the cubic is valid)
                    oc = tmp.tile([P, G], bf16, name="oc", tag="oc")
                    nc.vector.tensor_mul(oc, g_sb["o"], c_out)
                    c2 = tmp.tile([P, G], bf16, name="c2", tag="c2")
                    nc.vector.tensor_mul(c2, c_out, c_out)
                    w3 = tmp.tile([P, G], bf16, name="w3", tag="w3")
                    nc.vector.tensor_scalar(w3, c2, TANH3_A, 1.0, MULT,
                                            ADDOP)
                    nc.vector.tensor_mul(h_out, oc, w3)
                    return
                tt = tmp.tile([P, G], bf16, name="tt", tag="tt")
                nc.scalar.activation(out=tt, in_=c_out, func=TANH,
                                     bias=0.0, scale=1.0)
                nc.vector.tensor_mul(h_out, g_sb["o"], tt)

            def f_pass(xp_ap, hch_ap, cch_ap, fcs_out, hs_out, S,
                       h_dve=False, fc_gps=False):
                """f gates for S child cols; reduce fc and h by groups of 4.
                xp_ap: broadcast AP of the S//BR parent embed columns.
                h_dve: do the h child-sum on VectorE instead of GpSimd
                (used in the serial tail where GpSimd dispatch latency
                would sit on the critical path).
                fc_gps: do the fc child-sum on GpSimd (used for the bulk
                leaf-level work to offload the saturated VectorE)."""
                ps = psum.tile([P, GMAX], f32, name="ps_f", tag="ps")
                matmul_group(ps, "fx", xp_ap, "fh", hch_ap, G=S)
                f = tmp.tile([P, S], bf16, name="g_f", tag="g_f", bufs=1)
                nc.scalar.activation(out=f, in_=ps[:, :S], func=SIG,
                                     bias=bias_ap("f"), scale=1.0)
                fc = tmp.tile([P, S], bf16, name="fc", tag="fc", bufs=1)
                nc.vector.tensor_mul(fc, f, cch_ap)
                # pairwise reduce x4 -> fcs_out
                feng = nc.gpsimd if fc_gps else nc.vector
                fcv = fc.rearrange("p (a two) -> p a two", two=2)
                rt = tmp.tile([P, S // 2], bf16, name="rt", tag="rt",
                              bufs=1)
                feng.tensor_tensor(rt, fcv[:, :, 0], fcv[:, :, 1], ADD)
                rtv = rt.rearrange("p (a two) -> p a two", two=2)
                feng.tensor_tensor(fcs_out, rtv[:, :, 0], rtv[:, :, 1], ADD)
                # pairwise reduce h x4 -> hs_out
                eng = nc.vector if h_dve else nc.gpsimd
                hv = hch_ap.rearrange("p (a two) -> p a two", two=2)
                rt2 = tmp.tile([P, S // 2], bf16, name="rt2", tag="rt2",
                               bufs=1)
                eng.tensor_tensor(rt2, hv[:, :, 0], hv[:, :, 1], ADD)
                rt2v = rt2.rearrange("p (a two) -> p a two", two=2)
                eng.tensor_tensor(hs_out, rt2v[:, :, 0], rt2v[:, :, 1],
                                  ADD)

            def load_leaf(tag, a, b):
                t = stream.tile([P, b - a], bf16, name=tag, tag=tag, bufs=3)
                nc.sync.dma_start(out=t, in_=xT[:, a:b])
                return t

            # ================= leaves fused with level depth-2 ============
            l6 = depth - 2
            nchunks = nleaf // ch_leaf
            q = ch_leaf // BR          # level-l6 parents produced per chunk
            # Variable level-l6 group sizes (in parents): small groups at
            # the start (parent work becomes available early, filling the
            # pipeline ramp) and at the end (the drain telescopes through
            # small slices instead of one serial 2048-wide chain).
            if nchunks == 16 and q == 512:
                group_sizes = [1024, 2048, 2048, 2048, 512, 512]
            else:
                group_sizes = [min(GMAX, n[l6])] * (n[l6] // min(GMAX, n[l6]))
            assert sum(group_sizes) == n[l6]
            groups = []            # (p0, gq)
            p0 = 0
            for gq in group_sizes:
                groups.append((p0, gq))
                p0 += gq
            chunk_group = {}       # chunk idx -> (group idx, jq, cpq)
            ck = 0
            for gi, (p0, gq) in enumerate(groups):
                cpq = gq // q
                for jq in range(cpq):
                    chunk_group[ck] = (gi, jq, cpq)
                    ck += 1
            fcs = hs = None
            pending = None
            l5 = depth - 3
            l4 = depth - 4
            # level depth-3 / depth-4 child-sum accumulators
            fcs5 = acc.tile([P, n[l5]], bf16, name="fcs5", tag="fcs5", bufs=1)
            hs5 = acc.tile([P, n[l5]], bf16, name="hs5", tag="hs5", bufs=1)
            fcs4 = acc.tile([P, n[l4]], bf16, name="fcs4", tag="fcs4", bufs=1)
            hs4 = acc.tile([P, n[l4]], bf16, name="hs4", tag="hs4", bufs=1)

            def emit_l6_gates(hs_p, fcs_p, p0, gq):
                h6o = chk.tile([P, gq], bf16, name="h6o", tag="h6o")
                c6o = chk.tile([P, gq], bf16, name="c6o", tag="c6o")
                gate_pass(xtp[:, off[l6] + p0:off[l6] + p0 + gq],
                          hs_p, fcs_p, h6o, c6o, gq)
                return (h6o, c6o, p0, gq)

            def emit_l5_f(h6o, c6o, p0, gq, drain=False):
                # f-pass of level depth-3 over the group produced above
                a5, q5 = p0 // BR, gq // BR
                f_pass(xpar_bcast(l5, a5, q5), h6o, c6o,
                       fcs5[:, a5:a5 + q5], hs5[:, a5:a5 + q5], gq,
                       h_dve=drain)
                return (a5, q5)

            def emit_l5_gates(a5, q5):
                gate_pass(xtp[:, off[l5] + a5:off[l5] + a5 + q5],
                          hs5[:, a5:a5 + q5], fcs5[:, a5:a5 + q5],
                          hres[l5][:, a5:a5 + q5], cres[l5][:, a5:a5 + q5],
                          q5)
                return (a5, q5)

            def emit_l4_f(a5, q5, drain=False):
                # children: level depth-3 cols [a5, a5+q5)
                a4, q4 = a5 // BR, q5 // BR
                f_pass(xpar_bcast(l4, a4, q4),
                       hres[l5][:, a5:a5 + q5], cres[l5][:, a5:a5 + q5],
                       fcs4[:, a4:a4 + q4], hs4[:, a4:a4 + q4], q5,
                       h_dve=drain)

            pending_l5f = None
            pending_l5g = None
            pending_l4f = None

            def emit_leaf_f(h7_p, c7_p, kp):
                # f-pass of level l6 over chunk kp's children
                nonlocal fcs, hs, pending
                gi, jq, cpq = chunk_group[kp]
                gq = groups[gi][1]
                if jq == 0:
                    fcs = acc.tile([P, gq], bf16, name="fcs", tag="fcs")
                    hs = acc.tile([P, gq], bf16, name="hs", tag="hs")
                a6 = kp * q              # parent cols of this chunk (lvl l6)
                f_pass(xpar_bcast(l6, a6, q), h7_p, c7_p,
                       fcs[:, jq * q:(jq + 1) * q],
                       hs[:, jq * q:(jq + 1) * q], ch_leaf, fc_gps=True)
                if jq == cpq - 1:
                    pending = (hs, fcs) + groups[gi]

            def step_parent_pipeline(drain=False):
                """Advance each deferred parent-work stage by one item.
                Emission order is oldest-stage-first so ScalarE always has
                ready work queued ahead of fresh dependencies."""
                nonlocal pending, pending_l5f, pending_l5g, pending_l4f
                if pending_l4f is not None:
                    emit_l4_f(*pending_l4f, drain=drain)
                    pending_l4f = None
                if pending_l5g is not None:
                    pending_l4f = emit_l5_gates(*pending_l5g)
                    pending_l5g = None
                if pending_l5f is not None:
                    pending_l5g = emit_l5_f(*pending_l5f, drain=drain)
                    pending_l5f = None
                if pending is not None:
                    pending_l5f = emit_l6_gates(*pending)
                    pending = None

            pre = []
            for k in range(min(2, nchunks)):
                a = off[lleaf] + k * ch_leaf
                pre.append(load_leaf("xt", a, a + ch_leaf))
            load_parent_embeds(off[l6])
            for k in range(nchunks):
                xt7 = pre.pop(0)
                if k + 2 < nchunks:
                    a = off[lleaf] + (k + 2) * ch_leaf
                    pre.append(load_leaf("xt", a, a + ch_leaf))
                h7k = stream.tile([P, ch_leaf], bf16, name="hch", tag="hch")
                c7k = stream.tile([P, ch_leaf], bf16, name="cch", tag="cch")
                # first chunks: exact tanh on ScalarE (no parent work exists
                # yet to fill it); later chunks: cubic tanh on VectorE
                gate_pass(xt7, None, None, h7k, c7k, ch_leaf,
                          leaf_dve_tanh=(k >= 2))
                # deferred work from previous chunks/groups: ACT chews the
                # leaf sigmoids while PE runs these matmuls on old data
                step_parent_pipeline()
                emit_leaf_f(h7k, c7k, k)
            # drain the software pipeline
            while (pending is not None or pending_l5f is not None
                   or pending_l5g is not None or pending_l4f is not None):
                step_parent_pipeline(drain=True)

            # gates of level depth-4 (its child-sums are complete now)
            gate_pass(xtp[:, off[l4]:off[l4] + n[l4]], hs4, fcs4,
                      hres[l4], cres[l4], n[l4])

            # ================= levels depth-5 .. 0 ========================
            for l in range(depth - 5, -1, -1):
                npar, nch = n[l], n[l + 1]
                S = min(GMAX, nch)
                ngr = nch // S
                fcs = acc.tile([P, npar], bf16, name="fcs", tag="fcs")
                hs = acc.tile([P, npar], bf16, name="hs", tag="hs")
                for g in range(ngr):
                    a = g * S
                    hch = hres[l + 1][:, a:a + S]
                    cch = cres[l + 1][:, a:a + S]
                    qq = S // BR
                    f_pass(xpar_bcast(l, a // BR, qq), hch, cch,
                           fcs[:, g * qq:(g + 1) * qq],
                           hs[:, g * qq:(g + 1) * qq], S, h_dve=True)
                gate_pass(xtp[:, off[l]:off[l] + npar], hs, fcs,
                          hres[l], cres[l], npar)

            # ================= output head ================================
            ps = psum.tile([P, GMAX], f32, name="ps_out", tag="ps")
            nc.tensor.matmul(ps[:tpc, :nlbl], hres[0], woutT_sb,
                             start=True, stop=True)
            out_sb = tmp.tile([tpc, nlbl], f32, name="out_sb", tag="out_sb")
            nc.vector.tensor_add(out_sb, ps[:tpc, :nlbl], bout_sb)
            nc.sync.dma_start(out=out[:], in_=out_sb)
    lp.__exit__(None, None, None)

    if legalize:
        _legalize_waits(nc)
    return nc


def _prep_core_inputs(embeds, weights, tpc, depth, n_cores):
    """Host-side shard + transpose per core."""
    n, off, ntot = _levels(tpc, depth)
    T = tpc * n_cores
    counts = [T * BR**l for l in range(depth)]
    offsets = [0]
    for c in counts:
        offsets.append(offsets[-1] + c)

    common = dict(weights)
    in_maps = []
    import ml_dtypes
    bf16 = ml_dtypes.bfloat16
    for d in range(n_cores):
        shard = np.concatenate(
            [embeds[offsets[l] + tpc * d * BR**l:
                    offsets[l] + tpc * (d + 1) * BR**l] for l in range(depth)],
            axis=0)
        xT = np.ascontiguousarray(shard.T.astype(bf16))   # [P, ntot]
        m = {"xT": xT}
        m.update(common)
        in_maps.append(m)
    return in_maps


def _prep_weights(Wix, bix, Wih, Wfx, bfx, Wfh, Wox, box, Woh, Wux, bux, Wuh,
                  Wout, bout, tpc):
    import ml_dtypes
    f = np.float32
    bf = ml_dtypes.bfloat16
    # order must match build_nc's wnames: ix, ih, fx, fh, ox, oh, ux, uh
    wall = np.stack([Wix.T, Wih.T, Wfx.T, Wfh.T, Wox.T, Woh.T, Wux.T, Wuh.T],
                    axis=1)   # [128 (in-feat), 8, 128 (out-feat)]
    w = {
        "Wall": np.ascontiguousarray(wall, dtype=bf),
        "bias4": np.ascontiguousarray(
            np.stack([bix, bfx, box, bux], axis=1), dtype=f),
        "WoutT": np.ascontiguousarray(Wout.T, dtype=bf),
        "bout2": np.ascontiguousarray(np.tile(bout, (tpc, 1)), dtype=f),
    }
    return w


def _ensure_ntff_hook():
    """The RL container's antenv lacks axon_hooks; install a shim and
    register the ctypes NTFF profiler so trace=True works."""
    import types

    try:
        from antenv.axon_hooks import get_axon_ntff_profile_hook  # noqa
        return
    except ImportError:
        pass
    mod = types.ModuleType("antenv.axon_hooks")
    _h = [None]
    mod.set_axon_ntff_profile_hook = lambda h: _h.__setitem__(0, h)
    mod.get_axon_ntff_profile_hook = lambda: _h[0]
    sys.modules["antenv.axon_hooks"] = mod
    import antenv
    antenv.axon_hooks = mod
    try:
        from trn_agent_boot.trn_boot import _ntff_profile_via_ctypes
        h = _ntff_profile_via_ctypes("/opt/axon/libaxon_pjrt.so")
        if h is not None:
            mod.set_axon_ntff_profile_hook(h)
    except Exception:
        pass


def kernel(embeds, Wix, bix, Wih, Wfx, bfx, Wfh, Wox, box, Woh, Wux, bux, Wuh,
           Wout, bout, _trace=False):
    from concourse import bass_utils
    from concourse.bass_utils import run_bass_kernel_spmd

    if _trace:
        _ensure_ntff_hook()
        bass_utils.upload_artifacts = lambda d: d  # no S3 in this container

    n_cores = 8
    depth = 8
    T = 16
    tpc = T // n_cores

    embeds = np.asarray(embeds, dtype=np.float32)
    weights = _prep_weights(
        np.asarray(Wix), np.asarray(bix), np.asarray(Wih), np.asarray(Wfx),
        np.asarray(bfx), np.asarray(Wfh), np.asarray(Wox), np.asarray(box),
        np.asarray(Woh), np.asarray(Wux), np.asarray(bux), np.asarray(Wuh),
        np.asarray(Wout), np.asarray(bout), tpc)
    in_maps = _prep_core_inputs(embeds, weights, tpc, depth, n_cores)

    key = (tpc, depth)
    if key not in _NC_CACHE:
        _NC_CACHE[key] = build_nc(tpc=tpc, depth=depth)
    nc = _NC_CACHE[key]

    res = run_bass_kernel_spmd(nc, in_maps, core_ids=list(range(n_cores)),
                               trace=_trace)
    outs = np.concatenate([r["out"] for r in res.results], axis=0)
    if _trace:
        kernel.last_results = res
    return outs.astype(np.float32)


kernel.last_results = None


# revision 4
# speedup vs baseline: 1.2773x; 1.2773x over previous
"""BatchChildSumTreeLSTM Trainium2 kernel, v2.

Forest of T complete B-ary trees, level-major layout, leaves-first.
Sharded across 8 NeuronCores by tree (2 trees/core); recursion is
device-local.

v2 changes vs baseline:
  * `xb` (parent-embed-per-child) input eliminated: the f-gate's fx
    matmul streams the PARENT xT slice through the PE with a
    column-repeat (stride-0) access pattern, so no extra DRAM traffic
    or host-side np.repeat.
  * Parent-level xT (levels 0..depth-2, 10922 cols/core) resident in
    SBUF via one up-front DMA; only the leaf level streams in chunks.
  * All elementwise tensors bf16 (incl. cell state c): every VectorE
    tensor_tensor runs in 2x_1P packed mode, roughly halving DVE busy
    time vs the f32 baseline.  PSUM stays f32; per-gate biases applied
    in fp32 by ScalarE's free affine.
"""

import sys

if "/opt/trn_rl_repo" not in sys.path:
    sys.path.insert(0, "/opt/trn_rl_repo")

import numpy as np

P = 128          # feature dim == partitions
BR = 4           # branching factor
NLBL = 5

_NC_CACHE = {}


def _levels(tpc, depth):
    n = [tpc * BR**l for l in range(depth)]
    off = [0]
    for c in n:
        off.append(off[-1] + c)
    return n, off, off[-1]


def _legalize_waits(nc, max_waits=1):
    """This walrus build accepts at most one sync-wait command per
    instruction (any type).  Hoist excess waits onto same-engine NoOps
    inserted right before the instruction; engine program order makes
    this exactly equivalent."""
    import concourse.mybir as mybir

    n_nops = 0
    for fn in nc.m.functions:
        for blk in fn.blocks:
            new_insts = []
            for inst in blk.instructions:
                si = getattr(inst, "sync_info", None)
                if si is not None and si.on_wait and len(si.on_wait) > max_waits:
                    waits = list(si.on_wait)
                    hoist, keep = waits[:-max_waits], waits[-max_waits:]
                    eng = getattr(inst, "engine", None)
                    for j, w in enumerate(hoist):
                        nop = mybir.InstNoOp(
                            name=f"{inst.name}-wn{j}",
                            engine=eng,
                            bass_nofuse=True,
                        )
                        nop.sync_info = mybir.SyncInfo(on_wait=[w],
                                                       on_update=[])
                        new_insts.append(nop)
                        n_nops += 1
                    inst.sync_info = mybir.SyncInfo(
                        on_wait=keep, on_update=list(si.on_update))
                new_insts.append(inst)
            blk.instructions = new_insts
    return n_nops


def build_nc(tpc=2, depth=8, ch_leaf=2048, nlbl=NLBL, legalize=True):
    """Build the per-core Bass/Tile program (identical on all cores)."""
    import concourse.bass as bass
    import concourse.mybir as mybir
    import concourse.tile as tile

    f32 = mybir.dt.float32
    bf16 = mybir.dt.bfloat16
    AF = mybir.ActivationFunctionType
    ADD = mybir.AluOpType.add

    n, off, ntot = _levels(tpc, depth)
    lleaf = depth - 1
    nleaf = n[lleaf]
    npar_tot = off[lleaf]          # nodes in levels 0..depth-2
    ch_leaf = min(ch_leaf, nleaf)
    assert nleaf % ch_leaf == 0

    nc = bass.Bass()

    xT = nc.dram_tensor("xT", [P, ntot], bf16, kind="ExternalInput")
    wnames = ["ix", "ih", "fx", "fh", "ox", "oh", "ux", "uh"]
    wall = nc.dram_tensor("Wall", [P, len(wnames), P], bf16,
                          kind="ExternalInput")
    bias4 = nc.dram_tensor("bias4", [P, 4], f32, kind="ExternalInput")
    woutT = nc.dram_tensor("WoutT", [P, nlbl], bf16, kind="ExternalInput")
    bout = nc.dram_tensor("bout2", [tpc, nlbl], f32, kind="ExternalInput")
    out = nc.dram_tensor("out", [tpc, nlbl], f32, kind="ExternalOutput")

    assert depth >= 4

    SIG, TANH = AF.Sigmoid, AF.Tanh
    BIDX = {"i": 0, "f": 1, "o": 2, "u": 3}

    lp = nc.allow_low_precision("bf16 LSTM cell state; tol 2e-2")
    lp.__enter__()
    with tile.TileContext(nc) as tc:
        import contextlib
        with contextlib.ExitStack() as ctx:
            wp = ctx.enter_context(tc.tile_pool(name="wp", bufs=1))
            stream = ctx.enter_context(tc.tile_pool(name="stream", bufs=2))
            tmp = ctx.enter_context(tc.tile_pool(name="tmp", bufs=2))
            acc = ctx.enter_context(tc.tile_pool(name="acc", bufs=2))
            chk = ctx.enter_context(tc.tile_pool(name="chk", bufs=2))
            hcp = ctx.enter_context(tc.tile_pool(name="hcp", bufs=1))
            psum = ctx.enter_context(
                tc.tile_pool(name="psum", bufs=2, space="PSUM"))

            GMAX = 2048  # max columns per gate/f group (4 PSUM banks)

            # ---- constants: tiles allocated here, DMAs issued inside the
            # leaf section (first leaf chunk first, spread across engine
            # DGE queues so nothing serializes behind the big loads) ----
            wall_sb = wp.tile([P, len(wnames), P], bf16, name="wall_sb",
                              tag="wall_sb")
            wsb = {nm: wall_sb[:, j, :] for j, nm in enumerate(wnames)}
            bias_sb = wp.tile([P, 4], f32, name="bias_sb", tag="bias_sb")
            woutT_sb = wp.tile([P, nlbl], bf16, name="woutT_sb", tag="woutT_sb")
            bout_sb = wp.tile([tpc, nlbl], f32, name="bout_sb", tag="bout_sb")
            xtp = wp.tile([P, npar_tot], bf16, name="xtp", tag="xtp")

            def load_consts_early():
                nc.sync.dma_start(out=wall_sb, in_=wall[:])
                nc.scalar.dma_start(out=bias_sb, in_=bias4[:])

            def load_parent_embeds(l6_off):
                # level depth-2 slice first (needed by the first leaf
                # f-pass), the small upper levels + output consts after
                nc.sync.dma_start(out=xtp[:, l6_off:npar_tot],
                                  in_=xT[:, l6_off:npar_tot])
                nc.sync.dma_start(out=xtp[:, 0:l6_off], in_=xT[:, 0:l6_off])
                nc.sync.dma_start(out=woutT_sb, in_=woutT[:])
                nc.sync.dma_start(out=bout_sb, in_=bout[:])

            def bias_ap(g):
                i = BIDX[g]
                return bias_sb[:, i:i + 1]

            def xpar_bcast(l, a, m):
                """Parent-embed columns [a, a+m) of level l, each column
                repeated BR times -> free size m*BR (the on-chip `xb`)."""
                base = xtp[:, off[l] + a:off[l] + a + m]
                return base.unsqueeze(2).broadcast_to((P, m, BR))

            # persistent h/c for resident levels (leaves and level
            # depth-2 are streamed/consumed in flight, never stored whole)
            hres, cres = {}, {}
            for l in range(depth - 2):
                hres[l] = hcp.tile([P, n[l]], bf16, name=f"h{l}_sb",
                                   tag=f"h{l}_sb")
                cres[l] = hcp.tile([P, n[l]], bf16, name=f"c{l}_sb",
                                   tag=f"c{l}_sb")

            def matmul_group(ps, w0, rhs0, w1=None, rhs1=None, G=GMAX):
                """ps[:, :G] = w0.T@rhs0 (+ w1.T@rhs1).  512-col banks."""
                nb = (G + 511) // 512
                for b in range(nb):
                    s = b * 512
                    e = min(s + 512, G)
                    nc.tensor.matmul(ps[:, s:e], wsb[w0],
                                     _slice_free(rhs0, s, e),
                                     start=True, stop=(w1 is None))
                if w1 is not None:
                    for b in range(nb):
                        s = b * 512
                        e = min(s + 512, G)
                        nc.tensor.matmul(ps[:, s:e], wsb[w1],
                                         _slice_free(rhs1, s, e),
                                         start=False, stop=True)

            def _slice_free(ap, s, e):
                """Slice [s, e) of the flattened free dim of a 2d or
                broadcast 3d AP (whose last dim is the BR repeat)."""
                if len(ap.shape) == 2:
                    return ap[:, s:e]
                assert s % BR == 0 and e % BR == 0
                return ap[:, s // BR:e // BR, :]

            # minimax cubic for tanh on [-1,1] (leaf c = i*u is always in
            # (-1,1)); end-to-end error is indistinguishable from exact
            # tanh at bf16 precision, and it moves leaf tanh off ScalarE
            TANH3_A = -0.2487
            MULT = mybir.AluOpType.mult
            ADDOP = mybir.AluOpType.add

            def gate_pass(xt_ap, hs_ap, fcs_ap, h_out, c_out, G,
                          leaf_dve_tanh=True):
                """Compute i,u,o gates + c,h for G parent columns."""
                leaf = hs_ap is None
                g_sb = {}
                for gname, wx, wh, func in (("i", "ix", "ih", SIG),
                                            ("u", "ux", "uh", TANH),
                                            ("o", "ox", "oh", SIG)):
                    ps = psum.tile([P, GMAX], f32, name=f"ps_{gname}",
                                   tag="ps")
                    if leaf:
                        matmul_group(ps, wx, xt_ap, G=G)
                    else:
                        matmul_group(ps, wx, xt_ap, wh, hs_ap, G=G)
                    g = tmp.tile([P, G], bf16, name=f"g_{gname}",
                                 tag=f"g_{gname}")
                    nc.scalar.activation(out=g, in_=ps[:, :G], func=func,
                                         bias=bias_ap(gname), scale=1.0)
                    g_sb[gname] = g
                nc.vector.tensor_mul(c_out, g_sb["i"], g_sb["u"])
                if fcs_ap is not None:
                    nc.vector.tensor_add(c_out, c_out, fcs_ap)
                if leaf and leaf_dve_tanh:
                    # h = (o*c)*(1 + A*c^2) on VectorE (ScalarE is the
                    # kernel-wide bottleneck; |c|<1 so the cubic is valid).
                    # c2/w3 first: they depend only on c, while oc needs
                    # the o-gate activation — don't block the DVE FIFO.
                    c2 = tmp.tile([P, G], bf16, name="c2", tag="c2")
                    nc.vector.tensor_mul(c2, c_out, c_out)
                    w3 = tmp.tile([P, G], bf16, name="w3", tag="w3")
                    nc.vector.tensor_scalar(w3, c2, TANH3_A, 1.0, MULT,
                                            ADDOP)
                    oc = tmp.tile([P, G], bf16, name="oc", tag="oc")
                    nc.vector.tensor_mul(oc, g_sb["o"], c_out)
                    nc.vector.tensor_mul(h_out, oc, w3)
                    return
                tt = tmp.tile([P, G], bf16, name="tt", tag="tt")
                nc.scalar.activation(out=tt, in_=c_out, func=TANH,
                                     bias=0.0, scale=1.0)
                nc.vector.tensor_mul(h_out, g_sb["o"], tt)

            def f_pass(xp_ap, hch_ap, cch_ap, fcs_out, hs_out, S,
                       h_dve=False, fc_gps=False):
                """f gates for S child cols; reduce fc and h by groups of 4.
                xp_ap: broadcast AP of the S//BR parent embed columns.
                h_dve: do the h child-sum on VectorE instead of GpSimd
                (used in the serial tail where GpSimd dispatch latency
                would sit on the critical path).
                fc_gps: do the fc child-sum on GpSimd (used for the bulk
                leaf-level work to offload the saturated VectorE)."""
                ps = psum.tile([P, GMAX], f32, name="ps_f", tag="ps")
                matmul_group(ps, "fx", xp_ap, "fh", hch_ap, G=S)
                f = tmp.tile([P, S], bf16, name="g_f", tag="g_f", bufs=1)
                nc.scalar.activation(out=f, in_=ps[:, :S], func=SIG,
                                     bias=bias_ap("f"), scale=1.0)
                fc = tmp.tile([P, S], bf16, name="fc", tag="fc", bufs=1)
                nc.vector.tensor_mul(fc, f, cch_ap)
                # pairwise reduce x4 -> fcs_out
                feng = nc.gpsimd if fc_gps else nc.vector
                fcv = fc.rearrange("p (a two) -> p a two", two=2)
                rt = tmp.tile([P, S // 2], bf16, name="rt", tag="rt",
                              bufs=1)
                feng.tensor_tensor(rt, fcv[:, :, 0], fcv[:, :, 1], ADD)
                rtv = rt.rearrange("p (a two) -> p a two", two=2)
                feng.tensor_tensor(fcs_out, rtv[:, :, 0], rtv[:, :, 1], ADD)
                # pairwise reduce h x4 -> hs_out
                eng = nc.vector if h_dve else nc.gpsimd
                hv = hch_ap.rearrange("p (a two) -> p a two", two=2)
                rt2 = tmp.tile([P, S // 2], bf16, name="rt2", tag="rt2",
                               bufs=1)
                eng.tensor_tensor(rt2, hv[:, :, 0], hv[:, :, 1], ADD)
                rt2v = rt2.rearrange("p (a two) -> p a two", two=2)
                eng.tensor_tensor(hs_out, rt2v[:, :, 0], rt2v[:, :, 1],
                                  ADD)

            def load_leaf(tag, a, b, eng=None):
                t = stream.tile([P, b - a], bf16, name=tag, tag=tag, bufs=3)
                (eng or nc.sync).dma_start(out=t, in_=xT[:, a:b])
                return t

            # ================= leaves fused with level depth-2 ============
            l6 = depth - 2
            nchunks = nleaf // ch_leaf
            q = ch_leaf // BR          # level-l6 parents produced per chunk
            # Variable level-l6 group sizes (in parents): small groups at
            # the start (parent work becomes available early, filling the
            # pipeline ramp) and at the end (the drain telescopes through
            # small slices instead of one serial 2048-wide chain).
            if nchunks == 16 and q == 512:
                group_sizes = [1024, 2048, 2048, 2048, 512, 512]
            else:
                group_sizes = [min(GMAX, n[l6])] * (n[l6] // min(GMAX, n[l6]))
            assert sum(group_sizes) == n[l6]
            groups = []            # (p0, gq)
            p0 = 0
            for gq in group_sizes:
                groups.append((p0, gq))
                p0 += gq
            chunk_group = {}       # chunk idx -> (group idx, jq, cpq)
            ck = 0
            for gi, (p0, gq) in enumerate(groups):
                cpq = gq // q
                for jq in range(cpq):
                    chunk_group[ck] = (gi, jq, cpq)
                    ck += 1
            fcs = hs = None
            pending = None
            l5 = depth - 3
            l4 = depth - 4
            # level depth-3 / depth-4 child-sum accumulators
            fcs5 = acc.tile([P, n[l5]], bf16, name="fcs5", tag="fcs5", bufs=1)
            hs5 = acc.tile([P, n[l5]], bf16, name="hs5", tag="hs5", bufs=1)
            fcs4 = acc.tile([P, n[l4]], bf16, name="fcs4", tag="fcs4", bufs=1)
            hs4 = acc.tile([P, n[l4]], bf16, name="hs4", tag="hs4", bufs=1)

            def emit_l6_gates(hs_p, fcs_p, p0, gq):
                h6o = chk.tile([P, gq], bf16, name="h6o", tag="h6o")
                c6o = chk.tile([P, gq], bf16, name="c6o", tag="c6o")
                gate_pass(xtp[:, off[l6] + p0:off[l6] + p0 + gq],
                          hs_p, fcs_p, h6o, c6o, gq)
                return (h6o, c6o, p0, gq)

            def emit_l5_f(h6o, c6o, p0, gq, drain=False):
                # f-pass of level depth-3 over the group produced above
                a5, q5 = p0 // BR, gq // BR
                f_pass(xpar_bcast(l5, a5, q5), h6o, c6o,
                       fcs5[:, a5:a5 + q5], hs5[:, a5:a5 + q5], gq,
                       h_dve=drain)
                return (a5, q5)

            def emit_l5_gates(a5, q5):
                gate_pass(xtp[:, off[l5] + a5:off[l5] + a5 + q5],
                          hs5[:, a5:a5 + q5], fcs5[:, a5:a5 + q5],
                          hres[l5][:, a5:a5 + q5], cres[l5][:, a5:a5 + q5],
                          q5)
                return (a5, q5)

            def emit_l4_f(a5, q5, drain=False):
                # children: level depth-3 cols [a5, a5+q5)
                a4, q4 = a5 // BR, q5 // BR
                f_pass(xpar_bcast(l4, a4, q4),
                       hres[l5][:, a5:a5 + q5], cres[l5][:, a5:a5 + q5],
                       fcs4[:, a4:a4 + q4], hs4[:, a4:a4 + q4], q5,
                       h_dve=drain)

            pending_l5f = None
            pending_l5g = None
            pending_l4f = None

            def emit_leaf_f(h7_p, c7_p, kp):
                # f-pass of level l6 over chunk kp's children
                nonlocal fcs, hs, pending
                gi, jq, cpq = chunk_group[kp]
                gq = groups[gi][1]
                if jq == 0:
                    fcs = acc.tile([P, gq], bf16, name="fcs", tag="fcs")
                    hs = acc.tile([P, gq], bf16, name="hs", tag="hs")
                a6 = kp * q              # parent cols of this chunk (lvl l6)
                f_pass(xpar_bcast(l6, a6, q), h7_p, c7_p,
                       fcs[:, jq * q:(jq + 1) * q],
                       hs[:, jq * q:(jq + 1) * q], ch_leaf)
                if jq == cpq - 1:
                    pending = (hs, fcs) + groups[gi]

            def step_parent_pipeline(drain=False):
                """Advance each deferred parent-work stage by one item.
                Emission order is oldest-stage-first so ScalarE always has
                ready work queued ahead of fresh dependencies."""
                nonlocal pending, pending_l5f, pending_l5g, pending_l4f
                if pending_l4f is not None:
                    emit_l4_f(*pending_l4f, drain=drain)
                    pending_l4f = None
                if pending_l5g is not None:
                    pending_l4f = emit_l5_gates(*pending_l5g)
                    pending_l5g = None
                if pending_l5f is not None:
                    pending_l5g = emit_l5_f(*pending_l5f, drain=drain)
                    pending_l5f = None
                if pending is not None:
                    pending_l5f = emit_l6_gates(*pending)
                    pending = None

            a0 = off[lleaf]
            pre = [load_leaf("xt", a0, a0 + ch_leaf)]
            load_consts_early()
            if nchunks > 1:
                pre.append(load_leaf("xt", a0 + ch_leaf, a0 + 2 * ch_leaf,
                                     eng=nc.scalar))
            load_parent_embeds(off[l6])
            for k in range(nchunks):
                xt7 = pre.pop(0)
                if k + 2 < nchunks:
                    a = off[lleaf] + (k + 2) * ch_leaf
                    pre.append(load_leaf("xt", a, a + ch_leaf))
                h7k = stream.tile([P, ch_leaf], bf16, name="hch", tag="hch")
                c7k = stream.tile([P, ch_leaf], bf16, name="cch", tag="cch")
                # leaf tanh alternates between exact (ScalarE) and cubic
                # (VectorE) to balance the two near-saturated engines; the
                # first chunks use ScalarE (no parent work to fill it yet)
                gate_pass(xt7, None, None, h7k, c7k, ch_leaf,
                          leaf_dve_tanh=(k >= 2 and k % 4 != 3))
                # deferred work from previous chunks/groups: ACT chews the
                # leaf sigmoids while PE runs these matmuls on old data
                step_parent_pipeline()
                emit_leaf_f(h7k, c7k, k)
            # drain the software pipeline
            while (pending is not None or pending_l5f is not None
                   or pending_l5g is not None or pending_l4f is not None):
                step_parent_pipeline(drain=True)

            # gates of level depth-4 (its child-sums are complete now)
            gate_pass(xtp[:, off[l4]:off[l4] + n[l4]], hs4, fcs4,
                      hres[l4], cres[l4], n[l4])

            # ================= levels depth-5 .. 0 ========================
            for l in range(depth - 5, -1, -1):
                npar, nch = n[l], n[l + 1]
                S = min(GMAX, nch)
                ngr = nch // S
                fcs = acc.tile([P, npar], bf16, name="fcs", tag="fcs")
                hs = acc.tile([P, npar], bf16, name="hs", tag="hs")
                for g in range(ngr):
                    a = g * S
                    hch = hres[l + 1][:, a:a + S]
                    cch = cres[l + 1][:, a:a + S]
                    qq = S // BR
                    f_pass(xpar_bcast(l, a // BR, qq), hch, cch,
                           fcs[:, g * qq:(g + 1) * qq],
                           hs[:, g * qq:(g + 1) * qq], S, h_dve=True)
                gate_pass(xtp[:, off[l]:off[l] + npar], hs, fcs,
                          hres[l], cres[l], npar)

            # ================= output head ================================
            ps = psum.tile([P, GMAX], f32, name="ps_out", tag="ps")
            nc.tensor.matmul(ps[:tpc, :nlbl], hres[0], woutT_sb,
                             start=True, stop=True)
            out_sb = tmp.tile([tpc, nlbl], f32, name="out_sb", tag="out_sb")
            nc.vector.tensor_add(out_sb, ps[:tpc, :nlbl], bout_sb)
            nc.sync.dma_start(out=out[:], in_=out_sb)
    lp.__exit__(None, None, None)

    if legalize:
        _legalize_waits(nc)
    return nc


def _prep_core_inputs(embeds, weights, tpc, depth, n_cores):
    """Host-side shard + transpose per core."""
    n, off, ntot = _levels(tpc, depth)
    T = tpc * n_cores
    counts = [T * BR**l for l in range(depth)]
    offsets = [0]
    for c in counts:
        offsets.append(offsets[-1] + c)

    common = dict(weights)
    in_maps = []
    import ml_dtypes
    bf16 = ml_dtypes.bfloat16
    for d in range(n_cores):
        shard = np.concatenate(
            [embeds[offsets[l] + tpc * d * BR**l:
                    offsets[l] + tpc * (d + 1) * BR**l] for l in range(depth)],
            axis=0)
        xT = np.ascontiguousarray(shard.T.astype(bf16))   # [P, ntot]
        m = {"xT": xT}
        m.update(common)
        in_maps.append(m)
    return in_maps


def _prep_weights(Wix, bix, Wih, Wfx, bfx, Wfh, Wox, box, Woh, Wux, bux, Wuh,
                  Wout, bout, tpc):
    import ml_dtypes
    f = np.float32
    bf = ml_dtypes.bfloat16
    # order must match build_nc's wnames: ix, ih, fx, fh, ox, oh, ux, uh
    wall = np.stack([Wix.T, Wih.T, Wfx.T, Wfh.T, Wox.T, Woh.T, Wux.T, Wuh.T],
                    axis=1)   # [128 (in-feat), 8, 128 (out-feat)]
    w = {
        "Wall": np.ascontiguousarray(wall, dtype=bf),
        "bias4": np.ascontiguousarray(
            np.stack([bix, bfx, box, bux], axis=1), dtype=f),
        "WoutT": np.ascontiguousarray(Wout.T, dtype=bf),
        "bout2": np.ascontiguousarray(np.tile(bout, (tpc, 1)), dtype=f),
    }
    return w


def _ensure_ntff_hook():
    """The RL container's antenv lacks axon_hooks; install a shim and
    register the ctypes NTFF profiler so trace=True works."""
    import types

    try:
        from antenv.axon_hooks import get_axon_ntff_profile_hook  # noqa
        return
    except ImportError:
        pass
    mod = types.ModuleType("antenv.axon_hooks")
    _h = [None]
    mod.set_axon_ntff_profile_hook = lambda h: _h.__setitem__(0, h)
    mod.get_axon_ntff_profile_hook = lambda: _h[0]
    sys.modules["antenv.axon_hooks"] = mod
    import antenv
    antenv.axon_hooks = mod
    try:
        from trn_agent_boot.trn_boot import _ntff_profile_via_ctypes
        h = _ntff_profile_via_ctypes("/opt/axon/libaxon_pjrt.so")
        if h is not None:
            mod.set_axon_ntff_profile_hook(h)
    except Exception:
        pass


def kernel(embeds, Wix, bix, Wih, Wfx, bfx, Wfh, Wox, box, Woh, Wux, bux, Wuh,
           Wout, bout, _trace=False):
    from concourse import bass_utils
    from concourse.bass_utils import run_bass_kernel_spmd

    if _trace:
        _ensure_ntff_hook()
        bass_utils.upload_artifacts = lambda d: d  # no S3 in this container

    n_cores = 8
    depth = 8
    T = 16
    tpc = T // n_cores

    embeds = np.asarray(embeds, dtype=np.float32)
    weights = _prep_weights(
        np.asarray(Wix), np.asarray(bix), np.asarray(Wih), np.asarray(Wfx),
        np.asarray(bfx), np.asarray(Wfh), np.asarray(Wox), np.asarray(box),
        np.asarray(Woh), np.asarray(Wux), np.asarray(bux), np.asarray(Wuh),
        np.asarray(Wout), np.asarray(bout), tpc)
    in_maps = _prep_core_inputs(embeds, weights, tpc, depth, n_cores)

    key = (tpc, depth)
    if key not in _NC_CACHE:
        _NC_CACHE[key] = build_nc(tpc=tpc, depth=depth)
    nc = _NC_CACHE[key]

    res = run_bass_kernel_spmd(nc, in_maps, core_ids=list(range(n_cores)),
                               trace=_trace)
    outs = np.concatenate([r["out"] for r in res.results], axis=0)
    if _trace:
        kernel.last_results = res
    return outs.astype(np.float32)


kernel.last_results = None


# revision 5
# speedup vs baseline: 1.2845x; 1.0056x over previous
"""BatchChildSumTreeLSTM Trainium2 kernel, v2.

Forest of T complete B-ary trees, level-major layout, leaves-first.
Sharded across 8 NeuronCores by tree (2 trees/core); recursion is
device-local.

v2 changes vs baseline:
  * `xb` (parent-embed-per-child) input eliminated: the f-gate's fx
    matmul streams the PARENT xT slice through the PE with a
    column-repeat (stride-0) access pattern, so no extra DRAM traffic
    or host-side np.repeat.
  * Parent-level xT (levels 0..depth-2, 10922 cols/core) resident in
    SBUF via one up-front DMA; only the leaf level streams in chunks.
  * All elementwise tensors bf16 (incl. cell state c): every VectorE
    tensor_tensor runs in 2x_1P packed mode, roughly halving DVE busy
    time vs the f32 baseline.  PSUM stays f32; per-gate biases applied
    in fp32 by ScalarE's free affine.
"""

import sys

if "/opt/trn_rl_repo" not in sys.path:
    sys.path.insert(0, "/opt/trn_rl_repo")

import numpy as np

P = 128          # feature dim == partitions
BR = 4           # branching factor
NLBL = 5

_NC_CACHE = {}


def _levels(tpc, depth):
    n = [tpc * BR**l for l in range(depth)]
    off = [0]
    for c in n:
        off.append(off[-1] + c)
    return n, off, off[-1]


def _legalize_waits(nc, max_waits=1):
    """This walrus build accepts at most one sync-wait command per
    instruction (any type).  Hoist excess waits onto same-engine NoOps
    inserted right before the instruction; engine program order makes
    this exactly equivalent."""
    import concourse.mybir as mybir

    n_nops = 0
    for fn in nc.m.functions:
        for blk in fn.blocks:
            new_insts = []
            for inst in blk.instructions:
                si = getattr(inst, "sync_info", None)
                if si is not None and si.on_wait and len(si.on_wait) > max_waits:
                    waits = list(si.on_wait)
                    hoist, keep = waits[:-max_waits], waits[-max_waits:]
                    eng = getattr(inst, "engine", None)
                    for j, w in enumerate(hoist):
                        nop = mybir.InstNoOp(
                            name=f"{inst.name}-wn{j}",
                            engine=eng,
                            bass_nofuse=True,
                        )
                        nop.sync_info = mybir.SyncInfo(on_wait=[w],
                                                       on_update=[])
                        new_insts.append(nop)
                        n_nops += 1
                    inst.sync_info = mybir.SyncInfo(
                        on_wait=keep, on_update=list(si.on_update))
                new_insts.append(inst)
            blk.instructions = new_insts
    return n_nops


def build_nc(tpc=2, depth=8, ch_leaf=2048, nlbl=NLBL, legalize=True):
    """Build the per-core Bass/Tile program (identical on all cores)."""
    import concourse.bass as bass
    import concourse.mybir as mybir
    import concourse.tile as tile

    f32 = mybir.dt.float32
    bf16 = mybir.dt.bfloat16
    AF = mybir.ActivationFunctionType
    ADD = mybir.AluOpType.add

    n, off, ntot = _levels(tpc, depth)
    lleaf = depth - 1
    nleaf = n[lleaf]
    npar_tot = off[lleaf]          # nodes in levels 0..depth-2
    ch_leaf = min(ch_leaf, nleaf)
    assert nleaf % ch_leaf == 0

    nc = bass.Bass()

    xT = nc.dram_tensor("xT", [P, ntot], bf16, kind="ExternalInput")
    wnames = ["ix", "ih", "fx", "fh", "ox", "oh", "ux", "uh"]
    wall = nc.dram_tensor("Wall", [P, len(wnames), P], bf16,
                          kind="ExternalInput")
    bias4 = nc.dram_tensor("bias4", [P, 4], f32, kind="ExternalInput")
    woutT = nc.dram_tensor("WoutT", [P, nlbl], bf16, kind="ExternalInput")
    bout = nc.dram_tensor("bout2", [tpc, nlbl], f32, kind="ExternalInput")
    out = nc.dram_tensor("out", [tpc, nlbl], f32, kind="ExternalOutput")

    assert depth >= 4

    SIG, TANH = AF.Sigmoid, AF.Tanh
    BIDX = {"i": 0, "f": 1, "o": 2, "u": 3}

    lp = nc.allow_low_precision("bf16 LSTM cell state; tol 2e-2")
    lp.__enter__()
    with tile.TileContext(nc) as tc:
        import contextlib
        with contextlib.ExitStack() as ctx:
            wp = ctx.enter_context(tc.tile_pool(name="wp", bufs=1))
            stream = ctx.enter_context(tc.tile_pool(name="stream", bufs=2))
            tmp = ctx.enter_context(tc.tile_pool(name="tmp", bufs=2))
            acc = ctx.enter_context(tc.tile_pool(name="acc", bufs=2))
            chk = ctx.enter_context(tc.tile_pool(name="chk", bufs=2))
            hcp = ctx.enter_context(tc.tile_pool(name="hcp", bufs=1))
            psum = ctx.enter_context(
                tc.tile_pool(name="psum", bufs=2, space="PSUM"))

            GMAX = 2048  # max columns per gate/f group (4 PSUM banks)

            # ---- constants: tiles allocated here, DMAs issued inside the
            # leaf section (first leaf chunk first, spread across engine
            # DGE queues so nothing serializes behind the big loads) ----
            wall_sb = wp.tile([P, len(wnames), P], bf16, name="wall_sb",
                              tag="wall_sb")
            wsb = {nm: wall_sb[:, j, :] for j, nm in enumerate(wnames)}
            bias_sb = wp.tile([P, 4], f32, name="bias_sb", tag="bias_sb")
            woutT_sb = wp.tile([P, nlbl], bf16, name="woutT_sb", tag="woutT_sb")
            bout_sb = wp.tile([tpc, nlbl], f32, name="bout_sb", tag="bout_sb")
            xtp = wp.tile([P, npar_tot], bf16, name="xtp", tag="xtp")

            def load_consts_early():
                nc.sync.dma_start(out=wall_sb, in_=wall[:])
                nc.scalar.dma_start(out=bias_sb, in_=bias4[:])
                # pre-warm the ACT spline table set (~2.7us load) while the
                # first DMAs are still in flight
                warm = wp.tile([1, 2], f32, name="warm", tag="warm")
                nc.vector.memset(warm[:, 0:1], 0)
                nc.scalar.activation(out=warm[:, 1:2], in_=warm[:, 0:1],
                                     func=SIG, bias=0.0, scale=1.0)

            def load_parent_embeds(l6_off):
                # level depth-2 slice first (needed by the first leaf
                # f-pass), the small upper levels + output consts after
                nc.sync.dma_start(out=xtp[:, l6_off:npar_tot],
                                  in_=xT[:, l6_off:npar_tot])
                nc.sync.dma_start(out=xtp[:, 0:l6_off], in_=xT[:, 0:l6_off])
                nc.sync.dma_start(out=woutT_sb, in_=woutT[:])
                nc.sync.dma_start(out=bout_sb, in_=bout[:])

            def bias_ap(g):
                i = BIDX[g]
                return bias_sb[:, i:i + 1]

            def xpar_bcast(l, a, m):
                """Parent-embed columns [a, a+m) of level l, each column
                repeated BR times -> free size m*BR (the on-chip `xb`)."""
                base = xtp[:, off[l] + a:off[l] + a + m]
                return base.unsqueeze(2).broadcast_to((P, m, BR))

            # persistent h/c for resident levels (leaves and level
            # depth-2 are streamed/consumed in flight, never stored whole)
            hres, cres = {}, {}
            for l in range(depth - 2):
                hres[l] = hcp.tile([P, n[l]], bf16, name=f"h{l}_sb",
                                   tag=f"h{l}_sb")
                cres[l] = hcp.tile([P, n[l]], bf16, name=f"c{l}_sb",
                                   tag=f"c{l}_sb")

            def matmul_group(ps, w0, rhs0, w1=None, rhs1=None, G=GMAX):
                """ps[:, :G] = w0.T@rhs0 (+ w1.T@rhs1).  512-col banks."""
                nb = (G + 511) // 512
                for b in range(nb):
                    s = b * 512
                    e = min(s + 512, G)
                    nc.tensor.matmul(ps[:, s:e], wsb[w0],
                                     _slice_free(rhs0, s, e),
                                     start=True, stop=(w1 is None))
                if w1 is not None:
                    for b in range(nb):
                        s = b * 512
                        e = min(s + 512, G)
                        nc.tensor.matmul(ps[:, s:e], wsb[w1],
                                         _slice_free(rhs1, s, e),
                                         start=False, stop=True)

            def _slice_free(ap, s, e):
                """Slice [s, e) of the flattened free dim of a 2d or
                broadcast 3d AP (whose last dim is the BR repeat)."""
                if len(ap.shape) == 2:
                    return ap[:, s:e]
                assert s % BR == 0 and e % BR == 0
                return ap[:, s // BR:e // BR, :]

            # minimax cubic for tanh on [-1,1] (leaf c = i*u is always in
            # (-1,1)); end-to-end error is indistinguishable from exact
            # tanh at bf16 precision, and it moves leaf tanh off ScalarE
            TANH3_A = -0.2487
            MULT = mybir.AluOpType.mult
            ADDOP = mybir.AluOpType.add

            def gate_pass(xt_ap, hs_ap, fcs_ap, h_out, c_out, G,
                          leaf_dve_tanh=True):
                """Compute i,u,o gates + c,h for G parent columns."""
                leaf = hs_ap is None
                g_sb = {}
                for gname, wx, wh, func in (("i", "ix", "ih", SIG),
                                            ("u", "ux", "uh", TANH),
                                            ("o", "ox", "oh", SIG)):
                    ps = psum.tile([P, GMAX], f32, name=f"ps_{gname}",
                                   tag="ps")
                    if leaf:
                        matmul_group(ps, wx, xt_ap, G=G)
                    else:
                        matmul_group(ps, wx, xt_ap, wh, hs_ap, G=G)
                    g = tmp.tile([P, G], bf16, name=f"g_{gname}",
                                 tag=f"g_{gname}")
                    nc.scalar.activation(out=g, in_=ps[:, :G], func=func,
                                         bias=bias_ap(gname), scale=1.0)
                    g_sb[gname] = g
                nc.vector.tensor_mul(c_out, g_sb["i"], g_sb["u"])
                if fcs_ap is not None:
                    nc.vector.tensor_add(c_out, c_out, fcs_ap)
                if leaf and leaf_dve_tanh:
                    # h = (o*c)*(1 + A*c^2) on VectorE (ScalarE is the
                    # kernel-wide bottleneck; |c|<1 so the cubic is valid).
                    # c2/w3 first: they depend only on c, while oc needs
                    # the o-gate activation — don't block the DVE FIFO.
                    c2 = tmp.tile([P, G], bf16, name="c2", tag="c2")
                    nc.vector.tensor_mul(c2, c_out, c_out)
                    w3 = tmp.tile([P, G], bf16, name="w3", tag="w3")
                    nc.vector.tensor_scalar(w3, c2, TANH3_A, 1.0, MULT,
                                            ADDOP)
                    oc = tmp.tile([P, G], bf16, name="oc", tag="oc")
                    nc.vector.tensor_mul(oc, g_sb["o"], c_out)
                    nc.vector.tensor_mul(h_out, oc, w3)
                    return
                tt = tmp.tile([P, G], bf16, name="tt", tag="tt")
                nc.scalar.activation(out=tt, in_=c_out, func=TANH,
                                     bias=0.0, scale=1.0)
                nc.vector.tensor_mul(h_out, g_sb["o"], tt)

            def f_pass(xp_ap, hch_ap, cch_ap, fcs_out, hs_out, S,
                       h_dve=False):
                """f gates for S child cols; reduce fc and h by groups of 4.
                xp_ap: broadcast AP of the S//BR parent embed columns.
                h_dve: do the h child-sum on VectorE — emitted BEFORE the
                fc work so it completes early.  Used on each group's
                closing chunk (the next level's gate matmuls wait on hs;
                GpSimd's ~4us dispatch+run latency would stall ScalarE)
                and in the serial drain/tail."""
                ps = psum.tile([P, GMAX], f32, name="ps_f", tag="ps")
                matmul_group(ps, "fx", xp_ap, "fh", hch_ap, G=S)
                f = tmp.tile([P, S], bf16, name="g_f", tag="g_f", bufs=1)
                nc.scalar.activation(out=f, in_=ps[:, :S], func=SIG,
                                     bias=bias_ap("f"), scale=1.0)

                def h_reduce(eng):
                    hv = hch_ap.rearrange("p (a two) -> p a two", two=2)
                    rt2 = tmp.tile([P, S // 2], bf16, name="rt2", tag="rt2",
                                   bufs=1)
                    eng.tensor_tensor(rt2, hv[:, :, 0], hv[:, :, 1], ADD)
                    rt2v = rt2.rearrange("p (a two) -> p a two", two=2)
                    eng.tensor_tensor(hs_out, rt2v[:, :, 0], rt2v[:, :, 1],
                                      ADD)

                if h_dve:
                    h_reduce(nc.vector)
                fc = tmp.tile([P, S], bf16, name="fc", tag="fc", bufs=1)
                nc.vector.tensor_mul(fc, f, cch_ap)
                # pairwise reduce x4 -> fcs_out  (VectorE)
                fcv = fc.rearrange("p (a two) -> p a two", two=2)
                rt = tmp.tile([P, S // 2], bf16, name="rt", tag="rt",
                              bufs=1)
                nc.vector.tensor_add(rt, fcv[:, :, 0], fcv[:, :, 1])
                rtv = rt.rearrange("p (a two) -> p a two", two=2)
                nc.vector.tensor_add(fcs_out, rtv[:, :, 0], rtv[:, :, 1])
                if not h_dve:
                    h_reduce(nc.gpsimd)

            def load_leaf(tag, a, b, eng=None):
                t = stream.tile([P, b - a], bf16, name=tag, tag=tag, bufs=3)
                (eng or nc.sync).dma_start(out=t, in_=xT[:, a:b])
                return t

            # ================= leaves fused with level depth-2 ============
            l6 = depth - 2
            nchunks = nleaf // ch_leaf
            q = ch_leaf // BR          # level-l6 parents produced per chunk
            # Variable level-l6 group sizes (in parents): small groups at
            # the start (parent work becomes available early, filling the
            # pipeline ramp) and at the end (the drain telescopes through
            # small slices instead of one serial 2048-wide chain).
            if nchunks == 16 and q == 512:
                group_sizes = [1024, 2048, 2048, 2048, 512, 512]
            else:
                group_sizes = [min(GMAX, n[l6])] * (n[l6] // min(GMAX, n[l6]))
            assert sum(group_sizes) == n[l6]
            groups = []            # (p0, gq)
            p0 = 0
            for gq in group_sizes:
                groups.append((p0, gq))
                p0 += gq
            chunk_group = {}       # chunk idx -> (group idx, jq, cpq)
            ck = 0
            for gi, (p0, gq) in enumerate(groups):
                cpq = gq // q
                for jq in range(cpq):
                    chunk_group[ck] = (gi, jq, cpq)
                    ck += 1
            fcs = hs = None
            pending = None
            l5 = depth - 3
            l4 = depth - 4
            # level depth-3 / depth-4 child-sum accumulators
            fcs5 = acc.tile([P, n[l5]], bf16, name="fcs5", tag="fcs5", bufs=1)
            hs5 = acc.tile([P, n[l5]], bf16, name="hs5", tag="hs5", bufs=1)
            fcs4 = acc.tile([P, n[l4]], bf16, name="fcs4", tag="fcs4", bufs=1)
            hs4 = acc.tile([P, n[l4]], bf16, name="hs4", tag="hs4", bufs=1)

            def emit_l6_gates(hs_p, fcs_p, p0, gq):
                h6o = chk.tile([P, gq], bf16, name="h6o", tag="h6o")
                c6o = chk.tile([P, gq], bf16, name="c6o", tag="c6o")
                gate_pass(xtp[:, off[l6] + p0:off[l6] + p0 + gq],
                          hs_p, fcs_p, h6o, c6o, gq)
                return (h6o, c6o, p0, gq)

            def emit_l5_f(h6o, c6o, p0, gq, drain=False):
                # f-pass of level depth-3 over the group produced above
                a5, q5 = p0 // BR, gq // BR
                f_pass(xpar_bcast(l5, a5, q5), h6o, c6o,
                       fcs5[:, a5:a5 + q5], hs5[:, a5:a5 + q5], gq,
                       h_dve=drain)
                return (a5, q5)

            def emit_l5_gates(a5, q5):
                gate_pass(xtp[:, off[l5] + a5:off[l5] + a5 + q5],
                          hs5[:, a5:a5 + q5], fcs5[:, a5:a5 + q5],
                          hres[l5][:, a5:a5 + q5], cres[l5][:, a5:a5 + q5],
                          q5)
                return (a5, q5)

            def emit_l4_f(a5, q5, drain=False):
                # children: level depth-3 cols [a5, a5+q5)
                a4, q4 = a5 // BR, q5 // BR
                f_pass(xpar_bcast(l4, a4, q4),
                       hres[l5][:, a5:a5 + q5], cres[l5][:, a5:a5 + q5],
                       fcs4[:, a4:a4 + q4], hs4[:, a4:a4 + q4], q5,
                       h_dve=drain)

            pending_l5f = None
            pending_l5g = None
            pending_l4f = None

            def emit_leaf_f(h7_p, c7_p, kp):
                # f-pass of level l6 over chunk kp's children
                nonlocal fcs, hs, pending
                gi, jq, cpq = chunk_group[kp]
                gq = groups[gi][1]
                if jq == 0:
                    fcs = acc.tile([P, gq], bf16, name="fcs", tag="fcs")
                    hs = acc.tile([P, gq], bf16, name="hs", tag="hs")
                a6 = kp * q              # parent cols of this chunk (lvl l6)
                f_pass(xpar_bcast(l6, a6, q), h7_p, c7_p,
                       fcs[:, jq * q:(jq + 1) * q],
                       hs[:, jq * q:(jq + 1) * q], ch_leaf,
                       h_dve=(jq == cpq - 1))
                if jq == cpq - 1:
                    pending = (hs, fcs) + groups[gi]

            def step_parent_pipeline(drain=False):
                """Advance each deferred parent-work stage by one item.
                Emission order is oldest-stage-first so ScalarE always has
                ready work queued ahead of fresh dependencies."""
                nonlocal pending, pending_l5f, pending_l5g, pending_l4f
                if pending_l4f is not None:
                    emit_l4_f(*pending_l4f, drain=drain)
                    pending_l4f = None
                if pending_l5g is not None:
                    pending_l4f = emit_l5_gates(*pending_l5g)
                    pending_l5g = None
                if pending_l5f is not None:
                    pending_l5g = emit_l5_f(*pending_l5f, drain=drain)
                    pending_l5f = None
                if pending is not None:
                    pending_l5f = emit_l6_gates(*pending)
                    pending = None

            a0 = off[lleaf]
            pre = [load_leaf("xt", a0, a0 + ch_leaf)]
            load_consts_early()
            if nchunks > 1:
                pre.append(load_leaf("xt", a0 + ch_leaf, a0 + 2 * ch_leaf,
                                     eng=nc.scalar))
            load_parent_embeds(off[l6])
            for k in range(nchunks):
                xt7 = pre.pop(0)
                if k + 2 < nchunks:
                    a = off[lleaf] + (k + 2) * ch_leaf
                    pre.append(load_leaf("xt", a, a + ch_leaf))
                h7k = stream.tile([P, ch_leaf], bf16, name="hch", tag="hch")
                c7k = stream.tile([P, ch_leaf], bf16, name="cch", tag="cch")
                # leaf tanh alternates between exact (ScalarE) and cubic
                # (VectorE) to balance the two near-saturated engines; the
                # first chunks use ScalarE (no parent work to fill it yet)
                gate_pass(xt7, None, None, h7k, c7k, ch_leaf,
                          leaf_dve_tanh=(k >= 2 and k % 4 != 3))
                # deferred work from previous chunks/groups: ACT chews the
                # leaf sigmoids while PE runs these matmuls on old data
                step_parent_pipeline()
                emit_leaf_f(h7k, c7k, k)
            # drain the software pipeline
            while (pending is not None or pending_l5f is not None
                   or pending_l5g is not None or pending_l4f is not None):
                step_parent_pipeline(drain=True)

            # gates of level depth-4 (its child-sums are complete now)
            gate_pass(xtp[:, off[l4]:off[l4] + n[l4]], hs4, fcs4,
                      hres[l4], cres[l4], n[l4])

            # ================= levels depth-5 .. 0 ========================
            for l in range(depth - 5, -1, -1):
                npar, nch = n[l], n[l + 1]
                S = min(GMAX, nch)
                ngr = nch // S
                fcs = acc.tile([P, npar], bf16, name="fcs", tag="fcs")
                hs = acc.tile([P, npar], bf16, name="hs", tag="hs")
                for g in range(ngr):
                    a = g * S
                    hch = hres[l + 1][:, a:a + S]
                    cch = cres[l + 1][:, a:a + S]
                    qq = S // BR
                    f_pass(xpar_bcast(l, a // BR, qq), hch, cch,
                           fcs[:, g * qq:(g + 1) * qq],
                           hs[:, g * qq:(g + 1) * qq], S, h_dve=True)
                gate_pass(xtp[:, off[l]:off[l] + npar], hs, fcs,
                          hres[l], cres[l], npar)

            # ================= output head ================================
            ps = psum.tile([P, GMAX], f32, name="ps_out", tag="ps")
            nc.tensor.matmul(ps[:tpc, :nlbl], hres[0], woutT_sb,
                             start=True, stop=True)
            out_sb = tmp.tile([tpc, nlbl], f32, name="out_sb", tag="out_sb")
            nc.vector.tensor_add(out_sb, ps[:tpc, :nlbl], bout_sb)
            nc.sync.dma_start(out=out[:], in_=out_sb)
    lp.__exit__(None, None, None)

    if legalize:
        _legalize_waits(nc)
    return nc


def _prep_core_inputs(embeds, weights, tpc, depth, n_cores):
    """Host-side shard + transpose per core."""
    n, off, ntot = _levels(tpc, depth)
    T = tpc * n_cores
    counts = [T * BR**l for l in range(depth)]
    offsets = [0]
    for c in counts:
        offsets.append(offsets[-1] + c)

    common = dict(weights)
    in_maps = []
    import ml_dtypes
    bf16 = ml_dtypes.bfloat16
    for d in range(n_cores):
        shard = np.concatenate(
            [embeds[offsets[l] + tpc * d * BR**l:
                    offsets[l] + tpc * (d + 1) * BR**l] for l in range(depth)],
            axis=0)
        xT = np.ascontiguousarray(shard.T.astype(bf16))   # [P, ntot]
        m = {"xT": xT}
        m.update(common)
        in_maps.append(m)
    return in_maps


def _prep_weights(Wix, bix, Wih, Wfx, bfx, Wfh, Wox, box, Woh, Wux, bux, Wuh,
                  Wout, bout, tpc):
    import ml_dtypes
    f = np.float32
    bf = ml_dtypes.bfloat16
    # order must match build_nc's wnames: ix, ih, fx, fh, ox, oh, ux, uh
    wall = np.stack([Wix.T, Wih.T, Wfx.T, Wfh.T, Wox.T, Woh.T, Wux.T, Wuh.T],
                    axis=1)   # [128 (in-feat), 8, 128 (out-feat)]
    w = {
        "Wall": np.ascontiguousarray(wall, dtype=bf),
        "bias4": np.ascontiguousarray(
            np.stack([bix, bfx, box, bux], axis=1), dtype=f),
        "WoutT": np.ascontiguousarray(Wout.T, dtype=bf),
        "bout2": np.ascontiguousarray(np.tile(bout, (tpc, 1)), dtype=f),
    }
    return w


def _ensure_ntff_hook():
    """The RL container's antenv lacks axon_hooks; install a shim and
    register the ctypes NTFF profiler so trace=True works."""
    import types

    try:
        from antenv.axon_hooks import get_axon_ntff_profile_hook  # noqa
        return
    except ImportError:
        pass
    mod = types.ModuleType("antenv.axon_hooks")
    _h = [None]
    mod.set_axon_ntff_profile_hook = lambda h: _h.__setitem__(0, h)
    mod.get_axon_ntff_profile_hook = lambda: _h[0]
    sys.modules["antenv.axon_hooks"] = mod
    import antenv
    antenv.axon_hooks = mod
    try:
        from trn_agent_boot.trn_boot import _ntff_profile_via_ctypes
        h = _ntff_profile_via_ctypes("/opt/axon/libaxon_pjrt.so")
        if h is not None:
            mod.set_axon_ntff_profile_hook(h)
    except Exception:
        pass


def kernel(embeds, Wix, bix, Wih, Wfx, bfx, Wfh, Wox, box, Woh, Wux, bux, Wuh,
           Wout, bout, _trace=False):
    from concourse import bass_utils
    from concourse.bass_utils import run_bass_kernel_spmd

    if _trace:
        _ensure_ntff_hook()
        bass_utils.upload_artifacts = lambda d: d  # no S3 in this container

    n_cores = 8
    depth = 8
    T = 16
    tpc = T // n_cores

    embeds = np.asarray(embeds, dtype=np.float32)
    weights = _prep_weights(
        np.asarray(Wix), np.asarray(bix), np.asarray(Wih), np.asarray(Wfx),
        np.asarray(bfx), np.asarray(Wfh), np.asarray(Wox), np.asarray(box),
        np.asarray(Woh), np.asarray(Wux), np.asarray(bux), np.asarray(Wuh),
        np.asarray(Wout), np.asarray(bout), tpc)
    in_maps = _prep_core_inputs(embeds, weights, tpc, depth, n_cores)

    key = (tpc, depth)
    if key not in _NC_CACHE:
        _NC_CACHE[key] = build_nc(tpc=tpc, depth=depth)
    nc = _NC_CACHE[key]

    res = run_bass_kernel_spmd(nc, in_maps, core_ids=list(range(n_cores)),
                               trace=_trace)
    outs = np.concatenate([r["out"] for r in res.results], axis=0)
    if _trace:
        kernel.last_results = res
    return outs.astype(np.float32)


kernel.last_results = None
